# revision 1
# baseline (speedup 1.0000x reference)
"""GCNConv kernel for 8 Trainium2 NeuronCores.

Math: out = CSR_neighbor_sum(X @ W) == (CSR_neighbor_sum(X)) @ W
(the unweighted neighbor sum commutes with the right-multiplication by W).

Strategy (hardcoded for N=100000 nodes, degree 16, D=128, 8 cores):
  - Shard output nodes across 8 cores (12500 rows each); replicate X and W.
  - Per core, per 128-node tile: 16 indirect-DMA gathers (one per edge slot)
    each fetch 128 rows of X (one row per SBUF partition, int32 indices),
    a DVE binary tree sums the 16 gathered tiles, PE transposes the sum and
    multiplies by W, and the result is stored to that core's output shard.
  - Host only reshapes/shards the int32 index tensor and concatenates the
    8 output shards.
"""

import numpy as np

N_NODES = 100000
DEG = 16
D = 128
N_CORES = 8
NODES_PER_CORE = N_NODES // N_CORES  # 12500
P = 128  # SBUF partitions / nodes per tile

_CACHE = {}


def _build_nc():
    """Construct and compile the per-core Bass program (SPMD: same NEFF on
    all 8 cores; only the input tensors differ)."""
    import concourse.bass as bass
    import concourse.mybir as mybir
    from concourse import bacc
    from concourse.tile import TileContext
    from concourse.masks import make_identity

    n_tiles = (NODES_PER_CORE + P - 1) // P  # 98
    tail = NODES_PER_CORE - (n_tiles - 1) * P  # 84

    nc = bacc.Bacc("TRN2", target_bir_lowering=False, debug=False,
                   enable_asserts=True, num_devices=N_CORES)
    X = nc.dram_tensor("X", [N_NODES, D], mybir.dt.float32, kind="ExternalInput")
    W = nc.dram_tensor("W", [D, D], mybir.dt.float32, kind="ExternalInput")
    idx = nc.dram_tensor("idx", [NODES_PER_CORE, DEG], mybir.dt.int32,
                         kind="ExternalInput")
    out = nc.dram_tensor("out", [NODES_PER_CORE, D], mybir.dt.float32,
                         kind="ExternalOutput")

    with TileContext(nc) as tc:
        with (
            tc.tile_pool(name="const", bufs=1) as cpool,
            tc.tile_pool(name="idxp", bufs=3) as ipool,
            tc.tile_pool(name="gp", bufs=2) as gpool,
            tc.tile_pool(name="yp", bufs=3) as ypool,
            tc.tile_pool(name="op", bufs=3) as opool,
            tc.tile_pool(name="ps", bufs=4, space="PSUM") as pspool,
        ):
            w_sb = cpool.tile([D, D], mybir.dt.float32)
            nc.sync.dma_start(out=w_sb[:], in_=W[:])
            ident = cpool.tile([P, P], mybir.dt.float32)
            make_identity(nc, ident[:])

            for t in range(n_tiles):
                nt = P if t < n_tiles - 1 else tail
                r0 = t * P
                idx_sb = ipool.tile([P, DEG], mybir.dt.int32, tag="idx")
                nc.sync.dma_start(out=idx_sb[:nt, :], in_=idx[r0:r0 + nt, :])

                # 16 gathers: g_e[p, :] = X[idx[p, e], :]
                gs = []
                for e in range(DEG):
                    g = gpool.tile([P, D], mybir.dt.float32, tag=f"g{e}")
                    nc.gpsimd.indirect_dma_start(
                        out=g[:nt, :],
                        out_offset=None,
                        in_=X[:],
                        in_offset=bass.IndirectOffsetOnAxis(
                            ap=idx_sb[:nt, e:e + 1], axis=0),
                    )
                    gs.append(g)

                # binary-tree sum of the 16 gathered tiles (DVE)
                stride = 1
                while stride < DEG:
                    for a in range(0, DEG, 2 * stride):
                        nc.vector.tensor_add(
                            out=gs[a][:nt, :],
                            in0=gs[a][:nt, :],
                            in1=gs[a + stride][:nt, :],
                        )
                    stride *= 2
                y = gs[0]  # [nt, D] neighbor sum

                # transpose Y then multiply by W:  out = (Y^T)^T @ W = Y @ W
                yt_ps = pspool.tile([D, P], mybir.dt.float32, tag="yt")
                nc.tensor.transpose(out=yt_ps[:, :nt], in_=y[:nt, :],
                                    identity=ident[:nt, :nt])
                yt_sb = ypool.tile([D, P], mybir.dt.float32, tag="yt_sb")
                nc.vector.tensor_copy(out=yt_sb[:, :nt], in_=yt_ps[:, :nt])

                o_ps = pspool.tile([P, D], mybir.dt.float32, tag="ops")
                nc.tensor.matmul(out=o_ps[:nt, :], lhsT=yt_sb[:, :nt],
                                 rhs=w_sb[:], start=True, stop=True)
                o_sb = opool.tile([P, D], mybir.dt.float32, tag="osb")
                nc.vector.tensor_copy(out=o_sb[:nt, :], in_=o_ps[:nt, :])
                nc.sync.dma_start(out=out[r0:r0 + nt, :], in_=o_sb[:nt, :])
    nc.compile()
    return nc


def _get_nc():
    if "nc" not in _CACHE:
        _CACHE["nc"] = _build_nc()
    return _CACHE["nc"]


def _edge_matrix(row_pointers, column_index):
    """Per-node [N_NODES, DEG] int32 target matrix from the CSR arrays.
    The reference graph is fixed-degree DEG; handle it with a fast path."""
    rp = np.asarray(row_pointers).astype(np.int64)
    ci = np.asarray(column_index).astype(np.int32)
    deg = np.diff(rp)
    if len(deg) == N_NODES and (deg == DEG).all():
        return ci.reshape(N_NODES, DEG)
    # general CSR fallback: pad/truncate each row to DEG entries; rows with
    # fewer than DEG edges repeat... cannot represent without a zero row, so
    # only uniform-degree graphs are supported by this kernel build.
    raise NotImplementedError("kernel compiled for fixed degree 16 CSR")


def kernel(X, weights, row_pointers, column_index, blockPartition,
           edgeToColumn, edgeToRow):
    from concourse.bass_utils import run_bass_kernel_spmd

    X = np.ascontiguousarray(np.asarray(X), dtype=np.float32)
    W = np.ascontiguousarray(np.asarray(weights), dtype=np.float32)
    edges = _edge_matrix(row_pointers, column_index)  # [N, 16] int32

    nc = _get_nc()
    in_maps = []
    for c in range(N_CORES):
        lo = c * NODES_PER_CORE
        in_maps.append({
            "X": X,
            "W": W,
            "idx": np.ascontiguousarray(edges[lo:lo + NODES_PER_CORE]),
        })
    res = run_bass_kernel_spmd(nc, in_maps, core_ids=list(range(N_CORES)))
    return np.concatenate([r["out"] for r in res.results], axis=0)


# revision 2
# speedup vs baseline: 3.0310x; 3.0310x over previous
"""GCNConv kernel for 8 Trainium2 NeuronCores.

Math: out = CSR_neighbor_sum(X @ W) == (CSR_neighbor_sum(X)) @ W
(the unweighted neighbor sum commutes with the right-multiplication by W),
so each core gathers+sums raw X rows and applies the small [128,128] weight
matmul afterwards — no X' materialization round-trip through HBM.

Sharding (hardcoded for N=100000 nodes, degree 16, D=128, 8 cores):
  - Output nodes sharded across 8 cores (12500 rows each); X, W replicated.
  - Per core, per 128-node tile: one indirect-DMA gather per edge slot
    (128 rows per instruction, one row per SBUF partition, int32 indices),
    a DVE binary tree sums the gathered tiles, PE transposes the sum and
    multiplies by W, and the result is stored to the core's output shard.
  - Host work is limited to index reshaping/sharding and output concat.

The gather table is X with one appended all-zero row; graphs whose CSR is
not uniform degree-16 are handled by padding each node's neighbor list to a
power-of-two width with the zero-row sentinel index.
"""

import numpy as np

N_NODES = 100000
DEG = 16
D = 128
N_CORES = 8
NODES_PER_CORE = N_NODES // N_CORES  # 12500
P = 128  # SBUF partitions / nodes per tile
SENTINEL = N_NODES  # index of the appended zero row

_CACHE = {}


def _build_nc(w_pad):
    """Construct and compile the per-core Bass program (SPMD: same NEFF on
    all 8 cores; only input tensor contents differ). `w_pad` is the padded
    (power-of-two) number of edge slots per node."""
    import concourse.bass as bass
    import concourse.mybir as mybir
    from concourse import bacc
    from concourse.tile import TileContext
    from concourse.masks import make_identity

    n_tiles = (NODES_PER_CORE + P - 1) // P  # 98
    tail = NODES_PER_CORE - (n_tiles - 1) * P  # 84

    nc = bacc.Bacc("TRN2", target_bir_lowering=False, debug=False,
                   enable_asserts=True, num_devices=N_CORES)
    X = nc.dram_tensor("X", [N_NODES + 1, D], mybir.dt.float32,
                       kind="ExternalInput")
    W = nc.dram_tensor("W", [D, D], mybir.dt.float32, kind="ExternalInput")
    idx = nc.dram_tensor("idx", [NODES_PER_CORE, w_pad], mybir.dt.int32,
                         kind="ExternalInput")
    out = nc.dram_tensor("out", [NODES_PER_CORE, D], mybir.dt.float32,
                         kind="ExternalOutput")

    with TileContext(nc) as tc:
        with (
            tc.tile_pool(name="const", bufs=1) as cpool,
            tc.tile_pool(name="idxp", bufs=3) as ipool,
            tc.tile_pool(name="gp", bufs=2) as gpool,
            tc.tile_pool(name="yp", bufs=3) as ypool,
            tc.tile_pool(name="op", bufs=3) as opool,
            tc.tile_pool(name="ps", bufs=4, space="PSUM") as pspool,
        ):
            w_sb = cpool.tile([D, D], mybir.dt.float32)
            nc.sync.dma_start(out=w_sb[:], in_=W[:])
            ident = cpool.tile([P, P], mybir.dt.float32)
            make_identity(nc, ident[:])

            for t in range(n_tiles):
                nt = P if t < n_tiles - 1 else tail
                r0 = t * P
                idx_sb = ipool.tile([P, w_pad], mybir.dt.int32, tag="idx")
                nc.sync.dma_start(out=idx_sb[:nt, :], in_=idx[r0:r0 + nt, :])

                # one gather per edge slot: g_e[p, :] = X[idx[p, e], :]
                gs = []
                for e in range(w_pad):
                    g = gpool.tile([P, D], mybir.dt.float32, tag=f"g{e}")
                    nc.gpsimd.indirect_dma_start(
                        out=g[:nt, :],
                        out_offset=None,
                        in_=X[:],
                        in_offset=bass.IndirectOffsetOnAxis(
                            ap=idx_sb[:nt, e:e + 1], axis=0),
                    )
                    gs.append(g)

                # binary-tree sum of the gathered tiles (DVE)
                stride = 1
                while stride < w_pad:
                    for a in range(0, w_pad, 2 * stride):
                        nc.vector.tensor_add(
                            out=gs[a][:nt, :],
                            in0=gs[a][:nt, :],
                            in1=gs[a + stride][:nt, :],
                        )
                    stride *= 2
                y = gs[0]  # [nt, D] neighbor sum

                # transpose Y then multiply by W: out = (Y^T)^T @ W = Y @ W
                yt_ps = pspool.tile([D, P], mybir.dt.float32, tag="yt")
                nc.tensor.transpose(out=yt_ps[:, :nt], in_=y[:nt, :],
                                    identity=ident[:nt, :nt])
                yt_sb = ypool.tile([D, P], mybir.dt.float32, tag="yt_sb")
                nc.vector.tensor_copy(out=yt_sb[:, :nt], in_=yt_ps[:, :nt])

                o_ps = pspool.tile([P, D], mybir.dt.float32, tag="ops")
                nc.tensor.matmul(out=o_ps[:nt, :], lhsT=yt_sb[:, :nt],
                                 rhs=w_sb[:], start=True, stop=True)
                o_sb = opool.tile([P, D], mybir.dt.float32, tag="osb")
                nc.vector.tensor_copy(out=o_sb[:nt, :], in_=o_ps[:nt, :])
                nc.sync.dma_start(out=out[r0:r0 + nt, :], in_=o_sb[:nt, :])
    nc.compile()
    return nc


def _get_nc(w_pad):
    if w_pad not in _CACHE:
        _CACHE[w_pad] = _build_nc(w_pad)
    return _CACHE[w_pad]


def _edge_matrix(row_pointers, column_index):
    """Per-node [N_NODES, w_pad] int32 neighbor matrix from the CSR arrays,
    padded with the zero-row sentinel. Fast path for uniform degree DEG."""
    rp = np.asarray(row_pointers).astype(np.int64)
    ci = np.asarray(column_index).astype(np.int32)
    deg = np.diff(rp)
    if len(deg) == N_NODES and (deg == DEG).all() and rp[0] == 0 \
            and rp[-1] == len(ci):
        return ci.reshape(N_NODES, DEG), DEG
    # general CSR: replicate reference semantics
    # (row of edge e = searchsorted(rp, e, 'right') - 1, clipped to valid)
    e = np.arange(len(ci), dtype=np.int64)
    rows = np.searchsorted(rp, e, side="right") - 1
    valid = (rows >= 0) & (rows < N_NODES)
    rows = rows[valid]
    cols = ci[valid]
    order = np.argsort(rows, kind="stable")
    rows, cols = rows[order], cols[order]
    counts = np.bincount(rows, minlength=N_NODES)
    w_max = int(counts.max()) if len(counts) else 1
    w_pad = 1
    while w_pad < max(w_max, 2):
        w_pad *= 2
    mat = np.full((N_NODES, w_pad), SENTINEL, dtype=np.int32)
    # slot within row for each (sorted) edge
    starts = np.zeros(N_NODES + 1, dtype=np.int64)
    np.cumsum(counts, out=starts[1:])
    slot = np.arange(len(rows)) - starts[rows]
    mat[rows, slot] = np.clip(cols, 0, N_NODES - 1)
    return mat, w_pad


def kernel(X, weights, row_pointers, column_index, blockPartition,
           edgeToColumn, edgeToRow):
    from concourse.bass_utils import run_bass_kernel_spmd

    X = np.asarray(X, dtype=np.float32)
    X_pad = np.vstack([X, np.zeros((1, D), dtype=np.float32)])
    W = np.ascontiguousarray(np.asarray(weights), dtype=np.float32)
    edges, w_pad = _edge_matrix(row_pointers, column_index)

    nc = _get_nc(w_pad)
    in_maps = []
    for c in range(N_CORES):
        lo = c * NODES_PER_CORE
        in_maps.append({
            "X": X_pad,
            "W": W,
            "idx": np.ascontiguousarray(edges[lo:lo + NODES_PER_CORE]),
        })
    res = run_bass_kernel_spmd(nc, in_maps, core_ids=list(range(N_CORES)))
    return np.concatenate([r["out"] for r in res.results], axis=0)


# revision 4
# speedup vs baseline: 3.2914x; 1.0859x over previous
"""GCNConv kernel for 8 Trainium2 NeuronCores.

Math: out = CSR_neighbor_sum(X @ W) == (CSR_neighbor_sum(X)) @ W
(the unweighted neighbor sum commutes with the right-multiplication by W),
so each core gathers+sums raw X rows and applies the small [128,128] weight
matmul afterwards — no X' materialization round-trip through HBM.

Sharding (hardcoded for N=100000 nodes, degree 16, D=128, 8 cores):
  - Output nodes sharded across 8 cores (12500 rows each); X, W replicated.
  - Per core, per 128-node tile: one indirect-DMA gather per edge slot
    (128 rows per instruction, one row per SBUF partition, int32 indices),
    a DVE binary tree sums the gathered tiles, PE transposes the sum and
    multiplies by W, and the result is stored to the core's output shard.
  - Host work is limited to index reshaping/sharding and output concat.

The gather table is X with one appended all-zero row; graphs whose CSR is
not uniform degree-16 are handled by padding each node's neighbor list to a
power-of-two width with the zero-row sentinel index.
"""

import time

import numpy as np

N_NODES = 100000
DEG = 16
D = 128
N_CORES = 8
NODES_PER_CORE = N_NODES // N_CORES  # 12500
P = 128  # SBUF partitions / nodes per tile
SENTINEL = N_NODES  # index of the appended zero row

_CACHE = {}


def _build_nc(w_pad):
    """Construct and compile the per-core Bass program (SPMD: same NEFF on
    all 8 cores; only input tensor contents differ). `w_pad` is the padded
    (power-of-two) number of edge slots per node."""
    import concourse.bass as bass
    import concourse.mybir as mybir
    from concourse import bacc
    from concourse.tile import TileContext
    from concourse.masks import make_identity

    n_tiles = (NODES_PER_CORE + P - 1) // P  # 98
    tail = NODES_PER_CORE - (n_tiles - 1) * P  # 84

    nc = bacc.Bacc("TRN2", target_bir_lowering=False, debug=False,
                   enable_asserts=True, num_devices=N_CORES)
    X = nc.dram_tensor("X", [N_NODES + 1, D], mybir.dt.float32,
                       kind="ExternalInput")
    W = nc.dram_tensor("W", [D, D], mybir.dt.float32, kind="ExternalInput")
    idx = nc.dram_tensor("idx", [NODES_PER_CORE, w_pad], mybir.dt.int32,
                         kind="ExternalInput")
    out = nc.dram_tensor("out", [NODES_PER_CORE, D], mybir.dt.float32,
                         kind="ExternalOutput")

    with TileContext(nc) as tc:
        with (
            tc.tile_pool(name="const", bufs=1) as cpool,
            tc.tile_pool(name="idxp", bufs=3) as ipool,
            tc.tile_pool(name="gp", bufs=2) as gpool,
            tc.tile_pool(name="yp", bufs=3) as ypool,
            tc.tile_pool(name="op", bufs=3) as opool,
            tc.tile_pool(name="ps", bufs=4, space="PSUM") as pspool,
        ):
            w_sb = cpool.tile([D, D], mybir.dt.float32)
            nc.sync.dma_start(out=w_sb[:], in_=W[:])
            ident = cpool.tile([P, P], mybir.dt.float32)
            make_identity(nc, ident[:])

            for t in range(n_tiles):
                nt = P if t < n_tiles - 1 else tail
                r0 = t * P
                idx_sb = ipool.tile([P, w_pad], mybir.dt.int32, tag="idx")
                nc.sync.dma_start(out=idx_sb[:nt, :], in_=idx[r0:r0 + nt, :])

                # one gather per edge slot: g_e[p, :] = X[idx[p, e], :]
                gs = []
                for e in range(w_pad):
                    g = gpool.tile([P, D], mybir.dt.float32, tag=f"g{e}")
                    nc.gpsimd.indirect_dma_start(
                        out=g[:nt, :],
                        out_offset=None,
                        in_=X[:],
                        in_offset=bass.IndirectOffsetOnAxis(
                            ap=idx_sb[:nt, e:e + 1], axis=0),
                    )
                    gs.append(g)

                # binary-tree sum of the gathered tiles (DVE)
                stride = 1
                while stride < w_pad:
                    for a in range(0, w_pad, 2 * stride):
                        nc.vector.tensor_add(
                            out=gs[a][:nt, :],
                            in0=gs[a][:nt, :],
                            in1=gs[a + stride][:nt, :],
                        )
                    stride *= 2
                y = gs[0]  # [nt, D] neighbor sum

                # transpose Y then multiply by W: out = (Y^T)^T @ W = Y @ W
                yt_ps = pspool.tile([D, P], mybir.dt.float32, tag="yt")
                nc.tensor.transpose(out=yt_ps[:, :nt], in_=y[:nt, :],
                                    identity=ident[:nt, :nt])
                yt_sb = ypool.tile([D, P], mybir.dt.float32, tag="yt_sb")
                nc.vector.tensor_copy(out=yt_sb[:, :nt], in_=yt_ps[:, :nt])

                o_ps = pspool.tile([P, D], mybir.dt.float32, tag="ops")
                nc.tensor.matmul(out=o_ps[:nt, :], lhsT=yt_sb[:, :nt],
                                 rhs=w_sb[:], start=True, stop=True)
                o_sb = opool.tile([P, D], mybir.dt.float32, tag="osb")
                nc.vector.tensor_copy(out=o_sb[:nt, :], in_=o_ps[:nt, :])
                nc.sync.dma_start(out=out[r0:r0 + nt, :], in_=o_sb[:nt, :])
    nc.compile()
    return nc


def _get_nc(w_pad):
    if w_pad not in _CACHE:
        _CACHE[w_pad] = _build_nc(w_pad)
    return _CACHE[w_pad]


def _edge_matrix(row_pointers, column_index):
    """Per-node [N_NODES, w_pad] int32 neighbor matrix from the CSR arrays,
    padded with the zero-row sentinel. Fast path for uniform degree DEG."""
    rp = np.asarray(row_pointers).astype(np.int64)
    ci = np.asarray(column_index).astype(np.int32)
    deg = np.diff(rp)
    if len(deg) == N_NODES and (deg == DEG).all() and rp[0] == 0 \
            and rp[-1] == len(ci):
        return ci.reshape(N_NODES, DEG), DEG
    # general CSR: replicate reference semantics
    # (row of edge e = searchsorted(rp, e, 'right') - 1, clipped to valid)
    e = np.arange(len(ci), dtype=np.int64)
    rows = np.searchsorted(rp, e, side="right") - 1
    valid = (rows >= 0) & (rows < N_NODES)
    rows = rows[valid]
    cols = ci[valid]
    order = np.argsort(rows, kind="stable")
    rows, cols = rows[order], cols[order]
    counts = np.bincount(rows, minlength=N_NODES)
    w_max = int(counts.max()) if len(counts) else 1
    w_pad = 1
    while w_pad < max(w_max, 2):
        w_pad *= 2
    mat = np.full((N_NODES, w_pad), SENTINEL, dtype=np.int32)
    # slot within row for each (sorted) edge
    starts = np.zeros(N_NODES + 1, dtype=np.int64)
    np.cumsum(counts, out=starts[1:])
    slot = np.arange(len(rows)) - starts[rows]
    mat[rows, slot] = np.clip(cols, 0, N_NODES - 1)
    return mat, w_pad


def kernel(X, weights, row_pointers, column_index, blockPartition,
           edgeToColumn, edgeToRow):
    from concourse.bass_utils import run_bass_kernel_spmd

    X = np.asarray(X, dtype=np.float32)
    X_pad = np.vstack([X, np.zeros((1, D), dtype=np.float32)])
    W = np.ascontiguousarray(np.asarray(weights), dtype=np.float32)
    edges, w_pad = _edge_matrix(row_pointers, column_index)

    nc = _get_nc(w_pad)
    in_maps = []
    for c in range(N_CORES):
        lo = c * NODES_PER_CORE
        in_maps.append({
            "X": X_pad,
            "W": W,
            "idx": np.ascontiguousarray(edges[lo:lo + NODES_PER_CORE]),
        })
    last_exc = None
    for _attempt in range(3):
        try:
            res = run_bass_kernel_spmd(nc, in_maps,
                                       core_ids=list(range(N_CORES)))
            break
        except Exception as exc:  # transient NRT/axon errors recover on retry
            last_exc = exc
            time.sleep(15)
    else:
        raise last_exc
    return np.concatenate([r["out"] for r in res.results], axis=0)


# revision 5
# speedup vs baseline: 3.3906x; 1.0301x over previous
"""GCNConv kernel for 8 Trainium2 NeuronCores.

Math: out = CSR_neighbor_sum(X @ W) == (CSR_neighbor_sum(X)) @ W
(the unweighted neighbor sum commutes with the right-multiplication by W),
so each core gathers+sums raw X rows and applies the small [128,128] weight
matmul afterwards — no X' materialization round-trip through HBM.

Sharding (hardcoded for N=100000 nodes, degree 16, D=128, 8 cores):
  - Output nodes sharded across 8 cores (12500 rows each); X, W replicated.
  - Per core, per 128-node tile: one indirect-DMA gather per edge slot
    (128 rows per instruction, one row per SBUF partition, int32 indices),
    a DVE binary tree sums the gathered tiles, PE transposes the sum and
    multiplies by W, and the result is stored to the core's output shard.
  - Host work is limited to index reshaping/sharding and output concat.

The gather table is X with one appended all-zero row; graphs whose CSR is
not uniform degree-16 are handled by padding each node's neighbor list to a
power-of-two width with the zero-row sentinel index.
"""

import time

import numpy as np

N_NODES = 100000
DEG = 16
D = 128
N_CORES = 8
NODES_PER_CORE = N_NODES // N_CORES  # 12500
P = 128  # SBUF partitions / nodes per tile
SENTINEL = N_NODES  # index of the appended zero row

_CACHE = {}


def _build_nc(w_pad):
    """Construct and compile the per-core Bass program (SPMD: same NEFF on
    all 8 cores; only input tensor contents differ). `w_pad` is the padded
    (power-of-two) number of edge slots per node."""
    import concourse.bass as bass
    import concourse.mybir as mybir
    from concourse import bacc
    from concourse.tile import TileContext
    from concourse.masks import make_identity

    n_tiles = (NODES_PER_CORE + P - 1) // P  # 98
    tail = NODES_PER_CORE - (n_tiles - 1) * P  # 84

    nc = bacc.Bacc("TRN2", target_bir_lowering=False, debug=False,
                   enable_asserts=True, num_devices=N_CORES,
                   dynamic_dma_scratch_size=65536)
    X = nc.dram_tensor("X", [N_NODES + 1, D], mybir.dt.float32,
                       kind="ExternalInput")
    W = nc.dram_tensor("W", [D, D], mybir.dt.float32, kind="ExternalInput")
    idx = nc.dram_tensor("idx", [NODES_PER_CORE, w_pad], mybir.dt.int32,
                         kind="ExternalInput")
    out = nc.dram_tensor("out", [NODES_PER_CORE, D], mybir.dt.float32,
                         kind="ExternalOutput")

    with TileContext(nc) as tc:
        with (
            tc.tile_pool(name="const", bufs=1) as cpool,
            tc.tile_pool(name="idxp", bufs=3) as ipool,
            tc.tile_pool(name="gp", bufs=3) as gpool,
            tc.tile_pool(name="yp", bufs=3) as ypool,
            tc.tile_pool(name="op", bufs=3) as opool,
            tc.tile_pool(name="ps", bufs=4, space="PSUM") as pspool,
        ):
            w_sb = cpool.tile([D, D], mybir.dt.float32)
            nc.sync.dma_start(out=w_sb[:], in_=W[:])
            ident = cpool.tile([P, P], mybir.dt.float32)
            make_identity(nc, ident[:])

            for t in range(n_tiles):
                nt = P if t < n_tiles - 1 else tail
                r0 = t * P
                idx_sb = ipool.tile([P, w_pad], mybir.dt.int32, tag="idx")
                nc.sync.dma_start(out=idx_sb[:nt, :], in_=idx[r0:r0 + nt, :])

                # one gather per edge slot: g_e[p, :] = X[idx[p, e], :]
                gs = []
                for e in range(w_pad):
                    g = gpool.tile([P, D], mybir.dt.float32, tag=f"g{e}")
                    nc.gpsimd.indirect_dma_start(
                        out=g[:nt, :],
                        out_offset=None,
                        in_=X[:],
                        in_offset=bass.IndirectOffsetOnAxis(
                            ap=idx_sb[:nt, e:e + 1], axis=0),
                    )
                    gs.append(g)

                # binary-tree sum of the gathered tiles (DVE)
                stride = 1
                while stride < w_pad:
                    for a in range(0, w_pad, 2 * stride):
                        nc.vector.tensor_add(
                            out=gs[a][:nt, :],
                            in0=gs[a][:nt, :],
                            in1=gs[a + stride][:nt, :],
                        )
                    stride *= 2
                y = gs[0]  # [nt, D] neighbor sum

                # transpose Y then multiply by W: out = (Y^T)^T @ W = Y @ W
                yt_ps = pspool.tile([D, P], mybir.dt.float32, tag="yt")
                nc.tensor.transpose(out=yt_ps[:, :nt], in_=y[:nt, :],
                                    identity=ident[:nt, :nt])
                yt_sb = ypool.tile([D, P], mybir.dt.float32, tag="yt_sb")
                nc.vector.tensor_copy(out=yt_sb[:, :nt], in_=yt_ps[:, :nt])

                o_ps = pspool.tile([P, D], mybir.dt.float32, tag="ops")
                nc.tensor.matmul(out=o_ps[:nt, :], lhsT=yt_sb[:, :nt],
                                 rhs=w_sb[:], start=True, stop=True)
                o_sb = opool.tile([P, D], mybir.dt.float32, tag="osb")
                nc.vector.tensor_copy(out=o_sb[:nt, :], in_=o_ps[:nt, :])
                nc.sync.dma_start(out=out[r0:r0 + nt, :], in_=o_sb[:nt, :])
    nc.compile()
    return nc


def _get_nc(w_pad):
    if w_pad not in _CACHE:
        _CACHE[w_pad] = _build_nc(w_pad)
    return _CACHE[w_pad]


def _edge_matrix(row_pointers, column_index):
    """Per-node [N_NODES, w_pad] int32 neighbor matrix from the CSR arrays,
    padded with the zero-row sentinel. Fast path for uniform degree DEG."""
    rp = np.asarray(row_pointers).astype(np.int64)
    ci = np.asarray(column_index).astype(np.int32)
    deg = np.diff(rp)
    if len(deg) == N_NODES and (deg == DEG).all() and rp[0] == 0 \
            and rp[-1] == len(ci):
        return ci.reshape(N_NODES, DEG), DEG
    # general CSR: replicate reference semantics
    # (row of edge e = searchsorted(rp, e, 'right') - 1, clipped to valid)
    e = np.arange(len(ci), dtype=np.int64)
    rows = np.searchsorted(rp, e, side="right") - 1
    valid = (rows >= 0) & (rows < N_NODES)
    rows = rows[valid]
    cols = ci[valid]
    order = np.argsort(rows, kind="stable")
    rows, cols = rows[order], cols[order]
    counts = np.bincount(rows, minlength=N_NODES)
    w_max = int(counts.max()) if len(counts) else 1
    w_pad = 1
    while w_pad < max(w_max, 2):
        w_pad *= 2
    mat = np.full((N_NODES, w_pad), SENTINEL, dtype=np.int32)
    # slot within row for each (sorted) edge
    starts = np.zeros(N_NODES + 1, dtype=np.int64)
    np.cumsum(counts, out=starts[1:])
    slot = np.arange(len(rows)) - starts[rows]
    mat[rows, slot] = np.clip(cols, 0, N_NODES - 1)
    return mat, w_pad


def kernel(X, weights, row_pointers, column_index, blockPartition,
           edgeToColumn, edgeToRow):
    from concourse.bass_utils import run_bass_kernel_spmd

    X = np.asarray(X, dtype=np.float32)
    X_pad = np.vstack([X, np.zeros((1, D), dtype=np.float32)])
    W = np.ascontiguousarray(np.asarray(weights), dtype=np.float32)
    edges, w_pad = _edge_matrix(row_pointers, column_index)

    nc = _get_nc(w_pad)
    in_maps = []
    for c in range(N_CORES):
        lo = c * NODES_PER_CORE
        in_maps.append({
            "X": X_pad,
            "W": W,
            "idx": np.ascontiguousarray(edges[lo:lo + NODES_PER_CORE]),
        })
    last_exc = None
    for _attempt in range(3):
        try:
            res = run_bass_kernel_spmd(nc, in_maps,
                                       core_ids=list(range(N_CORES)))
            break
        except Exception as exc:  # transient NRT/axon errors recover on retry
            last_exc = exc
            time.sleep(15)
    else:
        raise last_exc
    return np.concatenate([r["out"] for r in res.results], axis=0)
